# revision 18
# baseline (speedup 1.0000x reference)
"""Trainium2 Bass kernel for nn_CaptionDecoder (attention LSTM caption decoder).

Strategy (8 NeuronCores):
  Phase A: data-parallel over batch (8 batches/core) for the sequential
           attention+2-layer-LSTM recurrence. Produces top-layer hidden
           states hb for all 20 steps.
  Host:    gathers hb from the 8 cores, reassembles.
  Phase B: vocab-parallel logits projection: every core gets the full
           hb and a 3750-column slice of W_out; host concatenates.

Precision: bf16 matmuls with fp32 PSUM accumulation. Sigmoid is computed
as 0.5*(1+tanh(x/2)); the 0.5 factors are folded into pre-scaled weights
by keeping doubled states H2=2h, C2=2c on device. The C~ gate's tanh(x)
is computed as tanh(0.5*(2x)) by doubling its weight columns host-side,
so ONE activation call covers all four gates of a layer.

Per-step structure (all engines kept busy, minimal serial hops):
  dec_proj (PE) -> drain (ACT) -> transpose (PE->DVE)
  adds enc+bias+dec (DVE/Pool split) -> tanh (ACT) -> v-reduce (PE)
  exp (ACT) -> alpha scatter (2 DMAs) -> context + sums (PE)
  recip (DVE) -> scale (ACT) -> transpose (PE->DVE)
  gates0 (PE, 4 col-tiled groups) -> tanh (ACT, 1 call) -> pointwise
  (DVE+Pool) -> transpose -> gates1 -> ... -> hb slot write
"""

import numpy as np
import ml_dtypes

import concourse.bass as bass
import concourse.bacc as bacc
import concourse.mybir as mybir
import concourse.tile as tile
from concourse.bass import ts
from concourse.bass_utils import run_bass_kernel_spmd
from concourse.masks import make_identity

F32 = mybir.dt.float32
BF16 = mybir.dt.bfloat16
AF = mybir.ActivationFunctionType
ALU = mybir.AluOpType

B, TC, P, E, H, A, V = 64, 21, 196, 512, 512, 512, 30000
T = TC - 1            # 20 decode steps
NC = 8                # cores
BL = B // NC          # 8 batches per core
PPAD = 256            # padded attention positions per batch
NF = BL * PPAD // 128  # 16 position chunks for context matmul
BP = BL * P           # 1568 (b, p) columns per core
G4 = 4 * H            # 2048 stacked gates f,i,C,o
VSL = V // NC         # 3750 vocab columns per core
BT = B * T            # 1280 output rows


def _bf16(x):
    return np.ascontiguousarray(np.asarray(x), dtype=None).astype(ml_dtypes.bfloat16)


def _sub(ap, dims, extra_offset=0):
    """Custom free-dim access pattern on an AP, keeping its partition dim."""
    return bass.AP(ap.tensor, ap.offset + extra_offset,
                   [list(ap.ap[0])] + [list(d) for d in dims])


def _ap(ap, dims, extra_offset=0):
    """Fully custom AP (first entry is the partition dim)."""
    return bass.AP(ap.tensor, ap.offset + extra_offset,
                   [list(d) for d in dims])


def _pbcast(ap, dims, extra_offset=0):
    """Partition-broadcast (stride 0) custom AP."""
    return bass.AP(ap.tensor, ap.offset + extra_offset,
                   [[0, 128]] + [list(d) for d in dims])


# --------------------------------------------------------------------------
# Phase A module: the recurrence
# --------------------------------------------------------------------------

def build_phase_a(n_steps=T):
    nc = bacc.Bacc("TRN2", num_devices=NC, debug=False)

    def din(name, shape, dt=BF16):
        return nc.dram_tensor(name, shape, dt, kind="ExternalInput").ap()

    encT = din("encT", [4, 128, BP])          # encoder_out^T  [e-chk][e][(b,p)]
    encflat = din("encflat", [NF, 128, E])    # [(b,ppad) chk][row][e], 0-padded
    featT = din("featT", [4, 128, BL])
    wih2 = din("wih2", [4, 128, H])           # 2*W_ih
    wic2 = din("wic2", [4, 128, H])           # 2*W_ic
    bih2 = din("bih2", [1, H])
    bic2 = din("bic2", [1, H])
    wenc = din("wenc", [4, 128, A])
    biasad = din("biasad", [1, A])            # b_enc + b_dec
    wdech = din("wdech", [4, 128, A])         # 0.5*W_dec
    vcol = din("vcol", [4, 128, 1])
    weT = din("weT", [4, 128, T * BL])        # embeds^T, col = t*8+b
    wg0x = din("wg0x", [4, 128, G4])          # C-cols 2x
    bg0 = din("bg0", [1, G4])                 # C-cols 2x
    wg0c = din("wg0c", [4, 128, G4])          # C-cols 2x
    wg0h = din("wg0h", [4, 128, G4])          # 0.5*, C-cols 1x
    wg1a = din("wg1a", [4, 128, G4])          # 0.5*, C-cols 1x
    wg1b = din("wg1b", [4, 128, G4])          # 0.5*, C-cols 1x
    bg1 = din("bg1", [1, G4])                 # C-cols 2x

    hballT = nc.dram_tensor("hballT", [4, 128, T * BL], BF16,
                            kind="ExternalOutput").ap()
    wepart = nc.dram_tensor("wepart", [T * BL, G4], BF16).ap()

    with tile.TileContext(nc) as tc:
        with (
            tc.tile_pool(name="persist", bufs=1) as pp,
        ):
            def dma3(dst, src, n=4):  # dram [n,128,X] -> sbuf [128,n,X]
                for k in range(n):
                    nc.sync.dma_start(out=dst[:, k], in_=src[k])

            # ---- persistent weights / constants --------------------------
            sb_wdech = pp.tile([128, 4, A], BF16, tag="wdech")
            dma3(sb_wdech, wdech)
            sb_vcol = pp.tile([128, 4, 1], BF16, tag="vcol")
            dma3(sb_vcol, vcol)
            sb_biasad = pp.tile([1, A], BF16, tag="biasad")
            nc.sync.dma_start(out=sb_biasad[:], in_=biasad)
            sb_wg0c = pp.tile([128, 4, G4], BF16, tag="wg0c")
            dma3(sb_wg0c, wg0c)
            sb_wg0h = pp.tile([128, 4, G4], BF16, tag="wg0h")
            dma3(sb_wg0h, wg0h)
            sb_wg1a = pp.tile([128, 4, G4], BF16, tag="wg1a")
            dma3(sb_wg1a, wg1a)
            sb_wg1b = pp.tile([128, 4, G4], BF16, tag="wg1b")
            dma3(sb_wg1b, wg1b)
            sb_bg1 = pp.tile([1, G4], BF16, tag="bg1")
            nc.sync.dma_start(out=sb_bg1[:], in_=bg1)
            sb_encflat = pp.tile([128, NF, E], BF16, tag="encflat")
            dma3(sb_encflat, encflat, n=NF)

            i8b = pp.tile([8, 8], BF16, tag="i8b")
            make_identity(nc, i8b[:])
            ones_1x8 = pp.tile([1, 8], BF16, tag="o18")
            nc.vector.memset(ones_1x8[:], 1.0)
            ones_1x128 = pp.tile([1, 128], BF16, tag="o1128")
            nc.vector.memset(ones_1x128[:], 1.0)
            ones_row = pp.tile([1, 512], BF16, tag="orow")
            nc.vector.memset(ones_row[:], 1.0)
            ones_col = pp.tile([128, 1], BF16, tag="ocol")
            nc.vector.memset(ones_col[:], 1.0)
            one_1x1 = pp.tile([1, 1], BF16, tag="o11")
            nc.vector.memset(one_1x1[:], 1.0)

            # state
            C2a = pp.tile([8, H], F32, tag="C2a")
            C2b = pp.tile([8, H], F32, tag="C2b")
            hball_sb = pp.tile([128, 4, T * BL], BF16, tag="hball")
            nc.vector.memset(hball_sb[:], 0.0)

            # alpha block-column tile: column 17*b (+8) of chunk f holds
            # alpha for batch b=f//2; every other cell stays 0 forever.
            aB = pp.tile([128, 128], BF16, tag="aB")
            nc.vector.memset(aB[:], 0.0)

            encproj = pp.tile([128, 4, BP], BF16, tag="encproj")

            # ---- one-time section (own pools, freed before the loop) -----
            sp = tc.alloc_tile_pool(name="stream", bufs=1)
            ps_once = tc.alloc_tile_pool(name="ps_once", bufs=1, space="PSUM")

            sb_featT = sp.tile([128, 4, BL], BF16, tag="featT")
            dma3(sb_featT, featT)
            sb_wih2 = sp.tile([128, 4, H], BF16, tag="wih2")
            dma3(sb_wih2, wih2)
            sb_wic2 = sp.tile([128, 4, H], BF16, tag="wic2")
            dma3(sb_wic2, wic2)
            sb_bih2 = sp.tile([1, H], BF16, tag="bih2")
            nc.sync.dma_start(out=sb_bih2[:], in_=bih2)
            sb_bic2 = sp.tile([1, H], BF16, tag="bic2")
            nc.sync.dma_start(out=sb_bic2[:], in_=bic2)

            # h0 = 2*(features @ W_ih + b_ih), c0 likewise
            h0ps = ps_once.tile([8, H], F32, tag="small")
            for k in range(4):
                nc.tensor.matmul(h0ps[:], sb_featT[:, k], sb_wih2[:, k],
                                 start=(k == 0), stop=False)
            nc.tensor.matmul(h0ps[:], ones_1x8[:], sb_bih2[:],
                             start=False, stop=True)
            h0sb = sp.tile([8, H], BF16, tag="h0sb")
            nc.scalar.activation(out=h0sb[:], in_=h0ps[:], func=AF.Copy)

            c0ps = ps_once.tile([8, H], F32, tag="small2")
            for k in range(4):
                nc.tensor.matmul(c0ps[:], sb_featT[:, k], sb_wic2[:, k],
                                 start=(k == 0), stop=False)
            nc.tensor.matmul(c0ps[:], ones_1x8[:], sb_bic2[:],
                             start=False, stop=True)
            nc.vector.tensor_copy(C2a[:], c0ps[:])
            nc.vector.tensor_copy(C2b[:], c0ps[:])

            # initial transposed state H2aT/H2bT [128,4,8]
            tp_once = ps_once.tile([128, 32], BF16, tag="tp")
            for k in range(4):
                nc.tensor.transpose(tp_once[:, ts(k, 8)], h0sb[:, ts(k, 128)],
                                    i8b[:])
            H2aT0 = pp.tile([128, 4, 8], BF16, tag="H2aT0")
            H2bT0 = pp.tile([128, 4, 8], BF16, tag="H2bT0")
            nc.vector.tensor_copy(
                H2aT0[:].rearrange("p a b -> p (a b)"), tp_once[:])
            nc.vector.tensor_copy(
                H2bT0[:].rearrange("p a b -> p (a b)"), tp_once[:])
            H2aT_prev, H2bT_prev = H2aT0, H2bT0

            # ---- one-time: enc_projT ------------------------------------
            sb_encT = sp.tile([128, 4, BP], BF16, tag="encT")
            dma3(sb_encT, encT)
            sb_wenc = sp.tile([128, 4, A], BF16, tag="wenc")
            dma3(sb_wenc, wenc)
            QS = BP // 4  # 392
            for k in range(4):          # a-chunk
                for q in range(4):      # bp quarter
                    eps = ps_once.tile([128, QS], F32, tag="eps")
                    nc.tensor.matmul(
                        eps[:], sb_biasad[:, ts(k, 128)], ones_row[:, 0:QS],
                        start=True, stop=False)
                    for e in range(4):  # e-chunk
                        nc.tensor.matmul(
                            eps[:], sb_wenc[:, e, ts(k, 128)],
                            sb_encT[:, e, ts(q, QS)],
                            start=False, stop=(e == 3))
                    nc.scalar.copy(out=encproj[:, k, ts(q, QS)], in_=eps[:])

            # ---- one-time: wepart (kept in SBUF) ------------------------
            sb_weT = sp.tile([128, 4, T * BL], BF16, tag="weT")
            dma3(sb_weT, weT)
            sb_wg0x = sp.tile([128, 4, G4], BF16, tag="wg0x")
            dma3(sb_wg0x, wg0x)
            sb_bg0 = sp.tile([1, G4], BF16, tag="bg0")
            nc.sync.dma_start(out=sb_bg0[:], in_=bg0)
            for m, rows in ((0, 128), (1, 32)):
                wps = ps_once.tile([128, G4], F32, tag="big")
                for j in range(4):
                    nc.tensor.matmul(wps[:rows, ts(j, 512)],
                                     ones_1x128[:, :rows],
                                     sb_bg0[:, ts(j, 512)],
                                     start=True, stop=False)
                    for e in range(4):
                        nc.tensor.matmul(
                            wps[:rows, ts(j, 512)],
                            sb_weT[:, e, m * 128:m * 128 + rows],
                            sb_wg0x[:, e, ts(j, 512)],
                            start=False, stop=(e == 3))
                wsb = sp.tile([128, G4], BF16, tag="wepsb")
                nc.vector.tensor_copy(wsb[:rows], wps[:rows])
                nc.sync.dma_start(out=wepart[m * 128:m * 128 + rows],
                                  in_=wsb[:rows])

            sp.release()
            ps_once.release()

            # ---- step pools ---------------------------------------------
            lp = tc.alloc_tile_pool(name="lp", bufs=2)
            lpH = tc.alloc_tile_pool(name="lpH", bufs=2)
            ps_sc = tc.alloc_tile_pool(name="ps_sc", bufs=2, space="PSUM")
            ps_at = tc.alloc_tile_pool(name="ps_at", bufs=1, space="PSUM")
            ps_dc = tc.alloc_tile_pool(name="ps_dc", bufs=1, space="PSUM")
            ps_g = tc.alloc_tile_pool(name="ps_g", bufs=1, space="PSUM")

            # ================= the recurrent steps ========================
            for t in range(n_steps):
                # prefetch this step's word-embedding gate part
                wet = lp.tile([8, G4], BF16, tag="wet")
                nc.sync.dma_start(out=wet[:], in_=wepart[t * 8:(t + 1) * 8])

                # -- dec_proj = H2b @ (0.5*W_dec)   [8, A]
                dps = ps_dc.tile([8, 512], F32, tag="dc", name="dps")
                for k in range(4):
                    nc.tensor.matmul(dps[:], H2bT_prev[:, k],
                                     sb_wdech[:, k],
                                     start=(k == 0), stop=(k == 3),
                                     skip_group_check=True)
                dsb = lp.tile([8, A], BF16, tag="dsb")
                nc.vector.tensor_copy(dsb[:], dps[:])
                tpd = ps_at.tile([128, 128], BF16, tag="at", name="tpd")
                for k in range(4):
                    nc.tensor.transpose(tpd[:, ts(k, 8)], dsb[:, ts(k, 128)],
                                        i8b[:])
                decT = lp.tile([128, 4, 8], BF16, tag="decT")
                nc.vector.tensor_copy(
                    decT[:].rearrange("p a b -> p (a b)"), tpd[:, 0:32])

                # -- s = tanh(enc_proj + biasad + dec_proj)
                ssb = lp.tile([128, 4, BP], BF16, tag="ssb")
                for h in range(2):          # batch-halves for pipelining
                    for k in range(4):
                        hb0 = h * 4
                        nc.vector.tensor_tensor(
                            out=ssb[:, k, h * 784:(h + 1) * 784]
                            .rearrange("p (b q) -> p b q", b=4),
                            in0=encproj[:, k, h * 784:(h + 1) * 784]
                            .rearrange("p (b q) -> p b q", b=4),
                            in1=decT[:, k, hb0:hb0 + 4].unsqueeze(2)
                            .broadcast_to([128, 4, P]),
                            op=ALU.add)
                        nc.scalar.activation(
                            out=ssb[:, k, h * 784:(h + 1) * 784],
                            in_=ssb[:, k, h * 784:(h + 1) * 784],
                            func=AF.Tanh)

                # -- scores = v^T s per 392-quarter; exp; transpose into
                # diag columns of atp via K=1 outer-product matmuls
                exp_row = lp.tile([1, BP], BF16, tag="exprow")
                atp = ps_at.tile([128, 512], F32, tag="at", name="atp")
                for q in range(4):
                    scq = ps_sc.tile([8, 512], F32, tag="sc", name="scq")
                    for k in range(4):
                        nc.tensor.matmul(
                            scq[0:1, 0:392], sb_vcol[:, k],
                            ssb[:, k, q * 392:(q + 1) * 392],
                            start=(k == 0), stop=(k == 3))
                    nc.scalar.activation(
                        out=exp_row[0:1, q * 392:(q + 1) * 392],
                        in_=scq[0:1, 0:392], func=AF.Exp)
                    for bh in range(4):       # (b', half) within the quarter
                        b = 2 * q + bh // 2
                        h = bh % 2
                        w = 128 if h == 0 else 68
                        col = 17 * b + 8 * h
                        nc.tensor.matmul(
                            atp[0:w, col:col + 1],
                            exp_row[0:1, b * 196 + 128 * h:
                                    b * 196 + 128 * h + w],
                            one_1x1[:], start=True, stop=True,
                            skip_group_check=True)
                # copy the diag columns into aB (2 strided copies)
                nc.vector.tensor_copy(
                    _sub(aB[:], [[17, 8]]),
                    _sub(atp[:], [[17, 8]]))
                nc.vector.tensor_copy(
                    _sub(aB[0:68], [[17, 8]], extra_offset=8),
                    _sub(atp[0:68], [[17, 8]], extra_offset=8))

                # -- softmax sums (diag-slice matmul into spare dps cols)
                nc.tensor.matmul(dps[:, 400:401],
                                 _sub(aB[:], [[17, 8]]), ones_col[:],
                                 start=True, stop=False,
                                 skip_group_check=True)
                nc.tensor.matmul(dps[:, 400:401],
                                 _sub(aB[0:68], [[17, 8]], extra_offset=8),
                                 ones_col[0:68],
                                 start=False, stop=True,
                                 skip_group_check=True)
                rinv_col = lp.tile([8, 1], F32, tag="rinv")
                nc.vector.reciprocal(rinv_col[:], dps[:, 400:401])

                # -- context = alpha @ encoder_out
                cps = ps_at.tile([8, 512], F32, tag="at", name="cps")
                for f in range(NF):
                    nc.tensor.matmul(cps[:], aB[:, ts(f, 8)],
                                     sb_encflat[:, f],
                                     start=(f == 0), stop=(f == NF - 1),
                                     skip_group_check=True)
                csb = lp.tile([8, E], BF16, tag="csb")
                nc.vector.tensor_scalar_mul(csb[:], cps[:], rinv_col[:])
                tpc = ps_at.tile([128, 128], BF16, tag="at", name="tpc")
                for k in range(4):
                    nc.tensor.transpose(tpc[:, ts(k, 8)],
                                        csb[:, ts(k, 128)], i8b[:])
                ctxT = lp.tile([128, 4, 8], BF16, tag="ctxT")
                nc.vector.tensor_copy(
                    ctxT[:].rearrange("p a b -> p (a b)"), tpc[:, 0:32])

                # -- gates0 = wepart[t] + ctx @ Wg0c + H2a @ (0.5*Wg0h)
                pt0 = ps_g.tile([8, G4], F32, tag="g", name="pt0")
                for j in range(4):
                    nc.tensor.matmul(pt0[:, ts(j, 512)], i8b[:],
                                     wet[:, ts(j, 512)],
                                     start=True, stop=False,
                                     skip_group_check=True)
                    for k in range(4):
                        nc.tensor.matmul(pt0[:, ts(j, 512)],
                                         H2aT_prev[:, k],
                                         sb_wg0h[:, k, ts(j, 512)],
                                         start=False, stop=False,
                                         skip_group_check=True)
                    for k in range(4):
                        nc.tensor.matmul(pt0[:, ts(j, 512)],
                                         ctxT[:, k],
                                         sb_wg0c[:, k, ts(j, 512)],
                                         start=False, stop=(k == 3),
                                         skip_group_check=True)

                def pointwise(pt, C2, hout, tag):
                    """gates psum [8,2048] (f,i,C,o along free) -> H2' bf16"""
                    tsb = lp.tile([8, G4], BF16, tag=tag + "_t")
                    for g in range(4):     # f, i, C, o separately: pipelines
                        nc.scalar.activation(out=tsb[:, ts(g, H)],
                                             in_=pt[:, ts(g, H)],
                                             func=AF.Tanh, scale=0.5)
                    s1 = lp.tile([8, H], F32, tag=tag + "_s1")
                    nc.vector.scalar_tensor_tensor(
                        out=s1[:], in0=tsb[:, 0:H], scalar=1.0, in1=C2[:],
                        op0=ALU.add, op1=ALU.mult)
                    s2 = lp.tile([8, H], F32, tag=tag + "_s2")
                    nc.vector.scalar_tensor_tensor(
                        out=s2[:], in0=tsb[:, H:2 * H], scalar=1.0,
                        in1=tsb[:, 2 * H:3 * H],
                        op0=ALU.add, op1=ALU.mult)
                    nc.vector.scalar_tensor_tensor(
                        out=C2[:], in0=s1[:], scalar=0.5, in1=s2[:],
                        op0=ALU.mult, op1=ALU.add)
                    tch = lp.tile([8, H], BF16, tag=tag + "_tc")
                    nc.scalar.activation(out=tch[:], in_=C2[:],
                                         func=AF.Tanh, scale=0.5)
                    nc.vector.scalar_tensor_tensor(
                        out=hout, in0=tsb[:, 3 * H:4 * H], scalar=1.0,
                        in1=tch[:], op0=ALU.add, op1=ALU.mult)

                h2a = lp.tile([8, H], BF16, tag="h2a")
                pointwise(pt0, C2a, h2a[:], "l0")
                tpa = ps_at.tile([128, 128], BF16, tag="at", name="tpa")
                for k in range(4):
                    nc.tensor.transpose(tpa[:, ts(k, 8)],
                                        h2a[:, ts(k, 128)], i8b[:])
                H2aT_new = lpH.tile([128, 4, 8], BF16, tag="H2aT")
                nc.vector.tensor_copy(
                    H2aT_new[:].rearrange("p a b -> p (a b)"), tpa[:, 0:32])

                # -- gates1 = b_g1 + H2a @ (0.5*Wg1a) + H2b @ (0.5*Wg1b)
                pt1 = ps_g.tile([8, G4], F32, tag="g", name="pt1")
                for j in range(4):
                    nc.tensor.matmul(pt1[:, ts(j, 512)], ones_1x8[:],
                                     sb_bg1[:, ts(j, 512)],
                                     start=True, stop=False,
                                     skip_group_check=True)
                    for k in range(4):
                        nc.tensor.matmul(pt1[:, ts(j, 512)],
                                         H2bT_prev[:, k],
                                         sb_wg1b[:, k, ts(j, 512)],
                                         start=False, stop=False,
                                         skip_group_check=True)
                    for k in range(4):
                        nc.tensor.matmul(pt1[:, ts(j, 512)],
                                         H2aT_new[:, k],
                                         sb_wg1a[:, k, ts(j, 512)],
                                         start=False, stop=(k == 3),
                                         skip_group_check=True)

                h2b = lp.tile([8, H], BF16, tag="h2b")
                pointwise(pt1, C2b, h2b[:], "l1")
                tpb = ps_at.tile([128, 128], BF16, tag="at", name="tpb")
                for k in range(4):
                    nc.tensor.transpose(tpb[:, ts(k, 8)],
                                        h2b[:, ts(k, 128)], i8b[:])
                nc.vector.tensor_copy(
                    hball_sb[:, :, t * 8:(t + 1) * 8],
                    tpb[:, 0:32].rearrange("p (a b) -> p a b", a=4))

                H2aT_prev = H2aT_new
                H2bT_prev = hball_sb[:, :, t * 8:(t + 1) * 8]

            for k in range(4):
                nc.sync.dma_start(out=hballT[k], in_=hball_sb[:, k])
            ps_g.release()
            ps_dc.release()
            ps_at.release()
            ps_sc.release()
            lpH.release()
            lp.release()

    nc.compile()
    return nc


# --------------------------------------------------------------------------
# Phase B module: logits = H2b_all @ (0.5*W_out) + b_out
# --------------------------------------------------------------------------

def build_phase_b():
    nc = bacc.Bacc("TRN2", num_devices=NC, debug=False)
    hbT = nc.dram_tensor("hbT", [4, 128, BT], BF16, kind="ExternalInput").ap()
    wout = nc.dram_tensor("wout", [4, 128, VSL], BF16,
                          kind="ExternalInput").ap()
    bout = nc.dram_tensor("bout", [128, VSL], BF16, kind="ExternalInput").ap()
    logits = nc.dram_tensor("logits", [BT, VSL], BF16,
                            kind="ExternalOutput").ap()

    vtiles = [(v, min(512, VSL - v)) for v in range(0, VSL, 512)]

    with tile.TileContext(nc) as tc:
        with (
            tc.tile_pool(name="w", bufs=1) as wp,
            tc.tile_pool(name="l", bufs=4) as lp,
            tc.tile_pool(name="ps", bufs=2, space="PSUM") as ps,
        ):
            sb_hbT = wp.tile([128, 4, BT], BF16, tag="hbT")
            for k in range(4):
                nc.sync.dma_start(out=sb_hbT[:, k], in_=hbT[k])
            sb_wout = wp.tile([128, 4, VSL], BF16, tag="wout")
            for k in range(4):
                nc.sync.dma_start(out=sb_wout[:, k], in_=wout[k])
            sb_bout = wp.tile([128, VSL], BF16, tag="bout")
            nc.sync.dma_start(out=sb_bout[:], in_=bout)

            for m in range(BT // 128):
                for vg in range(2):      # two groups of 4 v-tiles
                    vts = vtiles[vg * 4:(vg + 1) * 4]
                    pts = [ps.tile([128, 512], F32, tag=f"acc{i}",
                                   name=f"pt{i}")
                           for i in range(len(vts))]
                    # k-outer so the stationary hbT chunk is reused across
                    # the 4 v-tiles (one ldweights per k instead of four)
                    for k in range(4):
                        for i, (v0, vw) in enumerate(vts):
                            nc.tensor.matmul(pts[i][:, :vw],
                                             sb_hbT[:, k, ts(m, 128)],
                                             sb_wout[:, k, v0:v0 + vw],
                                             start=(k == 0), stop=(k == 3))
                    for i, (v0, vw) in enumerate(vts):
                        ot = lp.tile([128, 512], BF16, tag=f"out{i % 2}")
                        nc.vector.tensor_tensor(
                            out=ot[:, :vw], in0=pts[i][:, :vw],
                            in1=sb_bout[:, v0:v0 + vw], op=ALU.add)
                        nc.sync.dma_start(out=logits[ts(m, 128), v0:v0 + vw],
                                          in_=ot[:, :vw])
    nc.compile()
    return nc


# --------------------------------------------------------------------------
# Host-side preparation + driver
# --------------------------------------------------------------------------

def _fold_c2(w):
    """Double the C~ gate columns (1024:1536) of a [*, 2048] gate weight."""
    w = np.array(w, dtype=np.float32, copy=True)
    w[..., 2 * H:3 * H] *= 2.0
    return w


def prep_phase_a_inputs(features, encoder_out, emb, W_enc, b_enc, W_dec, b_dec,
                        v_w, W_g0, b_g0, W_g1, b_g1, W_ih, b_ih, W_ic, b_ic,
                        captions):
    embeds = np.asarray(emb)[np.asarray(captions)[:, :T].astype(np.int64)]
    shared = {
        "wih2": _bf16(2.0 * np.asarray(W_ih).reshape(4, 128, H)),
        "wic2": _bf16(2.0 * np.asarray(W_ic).reshape(4, 128, H)),
        "bih2": _bf16(2.0 * np.asarray(b_ih).reshape(1, H)),
        "bic2": _bf16(2.0 * np.asarray(b_ic).reshape(1, H)),
        "wenc": _bf16(np.asarray(W_enc).reshape(4, 128, A)),
        "biasad": _bf16((np.asarray(b_enc) + np.asarray(b_dec))
                        .reshape(1, A)),
        "wdech": _bf16(0.5 * np.asarray(W_dec).reshape(4, 128, A)),
        "vcol": _bf16(np.asarray(v_w).reshape(4, 128, 1)),
        "wg0x": _bf16(_fold_c2(np.asarray(W_g0)[:E]).reshape(4, 128, G4)),
        "bg0": _bf16(_fold_c2(np.asarray(b_g0)).reshape(1, G4)),
        "wg0c": _bf16(_fold_c2(np.asarray(W_g0)[E:2 * E])
                      .reshape(4, 128, G4)),
        "wg0h": _bf16(_fold_c2(0.5 * np.asarray(W_g0)[2 * E:])
                      .reshape(4, 128, G4)),
        "wg1a": _bf16(_fold_c2(0.5 * np.asarray(W_g1)[:H])
                      .reshape(4, 128, G4)),
        "wg1b": _bf16(_fold_c2(0.5 * np.asarray(W_g1)[H:])
                      .reshape(4, 128, G4)),
        "bg1": _bf16(_fold_c2(np.asarray(b_g1)).reshape(1, G4)),
    }
    in_maps = []
    for c in range(NC):
        bs = slice(c * BL, (c + 1) * BL)
        enc = np.asarray(encoder_out)[bs]               # [8, 196, 512]
        encTn = enc.transpose(2, 0, 1).reshape(E, BL * P)
        encpad = np.zeros((BL, PPAD, E), np.float32)
        encpad[:, :P] = enc
        feat = np.asarray(features)[bs]
        we = embeds[bs]                                 # [8, T, E]
        m = dict(shared)
        m["encT"] = _bf16(encTn.reshape(4, 128, BL * P))
        m["encflat"] = _bf16(encpad.reshape(NF, 128, E))
        m["featT"] = _bf16(feat.T.reshape(4, 128, BL))
        m["weT"] = _bf16(we.transpose(2, 1, 0).reshape(4, 128, T * BL))
        in_maps.append(m)
    return in_maps


_CACHE = {}


def kernel(**inputs):
    inputs = {k: np.asarray(v) for k, v in inputs.items()}
    if "a" not in _CACHE:
        _CACHE["a"] = build_phase_a()
    if "b" not in _CACHE:
        _CACHE["b"] = build_phase_b()

    in_a = prep_phase_a_inputs(
        inputs["features"], inputs["encoder_out"], inputs["emb"],
        inputs["W_enc"], inputs["b_enc"], inputs["W_dec"], inputs["b_dec"],
        inputs["v_w"], inputs["W_g0"], inputs["b_g0"], inputs["W_g1"],
        inputs["b_g1"], inputs["W_ih"], inputs["b_ih"], inputs["W_ic"],
        inputs["b_ic"], inputs["captions"])
    ra = run_bass_kernel_spmd(_CACHE["a"], in_a, core_ids=list(range(NC)))

    # reassemble hb: column index b*T + t
    hbT_full = np.zeros((4, 128, BT), dtype=ml_dtypes.bfloat16)
    for c in range(NC):
        part = ra.results[c]["hballT"].reshape(4, 128, T, BL)
        for bl in range(BL):
            b = c * BL + bl
            hbT_full[:, :, b * T:(b + 1) * T] = part[:, :, :, bl]

    W_out = np.asarray(inputs["W_out"])
    b_out = np.asarray(inputs["b_out"])
    in_b = []
    for c in range(NC):
        vs = slice(c * VSL, (c + 1) * VSL)
        in_b.append({
            "hbT": hbT_full,
            "wout": _bf16(0.5 * W_out[:, vs].reshape(4, 128, VSL)),
            "bout": _bf16(np.broadcast_to(b_out[vs], (128, VSL))),
        })
    rb = run_bass_kernel_spmd(_CACHE["b"], in_b, core_ids=list(range(NC)))
    logits = np.concatenate(
        [rb.results[c]["logits"].astype(np.float32) for c in range(NC)],
        axis=1)
    return logits.reshape(B, T, V)
